# revision 23
# baseline (speedup 1.0000x reference)
"""Distributed k-NN retrieval kernel for Trainium2 (8 NeuronCores, SPMD).

Math (per the problem): w_i = 1 / (||q - k_i||^2 + delta) over 1M keys;
top-50 w; out = sum_j values[idx_j] * (w_j / sum_i w_i), shape [1, 64].

Strategy: shard keys row-wise across 8 cores (125000 rows each, padded to
126976 = 2 * 63488). Each core computes all shard NEGATED partial
distances nd = -(||k||^2 - 2 q.k) = -dist + ||q||^2 via fp32 matmuls on
the tensor engine (channels on partitions; top-k of nd == top-k of w
since w = 1/(dist + delta) is strictly decreasing in dist), and extracts
an exact local top-256 superset per 63488-row half with the GpSimd topk
instruction (vocab=63488 fits the ISA's u16 field and the ucode's
vocab > 50000 requirement). The partial sum of w is computed exactly
on-device: nd is spread to 128 partitions, dist+delta recovered with one
tensor_scalar, then DVE reciprocal (iterative divide, exact) + reduce.
The host gathers 8 * 512 candidate indices + partial sums, recomputes the
candidate weights exactly in fp32, and does the final top-50 weighted
gather-sum (tiny: O(4096)).

Device-side layout (per core):
  row r in [0, 126976), decomposed r = 63488*b + 3968*s16 + 496*g + f
    b   in {0,1}   : topk half          (psum slice s = 16*b + s16)
    s16 in [0,16)  : psum-slice within half
    g   in [0,8)   : row group (psum partition)
    f   in [0,496) : psum free column
  channel c = 16*Q + cq (quarter Q in [0,4), cq in [0,16))
  kt[16*g + cq, 15872*Q + 496*s + f] = keys_pad[r, c]
  Each psum slice [8, 496] accumulates 8 matmuls (4 quarters x {-k^2, +2qk}).
  topk vocab index within half b: v = 3968*p16 + fv = r - 63488*b.
"""

import sys

import numpy as np

for _p in ("/opt/trn_rl_repo", "/opt/pypackages"):
    if _p not in sys.path:
        sys.path.insert(0, _p)

DELTA = 0.001
QUERY_WIDTH = 50
N_TOTAL = 1_000_000
D = 64
NCORES = 8
SHARD = N_TOTAL // NCORES  # 125000
FREE = 496                 # psum free columns per slice
SROWS = 8 * FREE           # 3968 rows per psum slice
HALF = 16 * SROWS          # 63488 rows per topk half (= topk vocab)
RPAD = 2 * HALF            # 126976 padded rows per core
W = RPAD * D // 128        # 63488 columns of the transposed layout
QBLK = W // 4              # 15872 cols per channel-quarter block
NTILE = 16                 # DMA tiles, each covers 2 psum slices
TOPK_K = 256
PAD_VAL = 1.0e6


def _emit_topk(nc, out_ap, in_ap, tokens: int, vocab_size: int, k: int):
    """gpsimd.topk minus its SBTensorHandle isinstance-assert, which rejects
    symbolic TilePool handles (the lowering handles them fine).
    Constraints kept: vocab_size in (50000, 65536) (u16 ISA field AND the
    ucode crashes below 50000), vocab_size % 128 == 0, k == 256."""
    import concourse.bass_isa as bass_isa

    assert in_ap.dtype.name == "float32" and out_ap.dtype.name == "uint32"
    assert vocab_size % 128 == 0 and 50000 < vocab_size < 65536 and k == 256
    _in = nc.gpsimd.lower_ap(in_ap, for_isa=True)
    _out = nc.gpsimd.lower_ap(out_ap, for_isa=True)
    return nc.gpsimd.add_instruction(
        bass_isa.InstTopk(
            name=f"I-{nc.next_id()}",
            ins=[_in],
            outs=[_out],
            _tokens=tokens,
            _n=vocab_size,
            _k=k,
        )
    )


def _build_nc(bias_const: float):
    import concourse.bacc as bacc
    import concourse.mybir as mybir
    import concourse.tile as tile

    nc = bacc.Bacc(None, target_bir_lowering=False)

    kt = nc.dram_tensor("kt", [128, W], mybir.dt.float32, kind="ExternalInput")
    sel8 = nc.dram_tensor("sel8", [128, 8], mybir.dt.float32, kind="ExternalInput")
    qsel = nc.dram_tensor("qsel", [128, 32], mybir.dt.float32, kind="ExternalInput")
    out_topk = nc.dram_tensor(
        "out_topk", [16, 64], mybir.dt.uint32, kind="ExternalOutput"
    )
    out_wacc = nc.dram_tensor(
        "out_wacc", [128, 2], mybir.dt.float32, kind="ExternalOutput"
    )

    with tile.TileContext(nc) as tc:
        with (
            tc.tile_pool(name="consts", bufs=1) as consts,
            tc.tile_pool(name="kpool", bufs=3) as kpool,
            tc.tile_pool(name="sqpool", bufs=2) as sqpool,
            tc.tile_pool(name="wpool", bufs=1) as wpool,
            tc.tile_pool(name="psum", bufs=4, space="PSUM") as psum,
        ):
            sel8_sb = consts.tile([128, 8], mybir.dt.float32, tag="sel8")
            qsel_sb = consts.tile([128, 32], mybir.dt.float32, tag="qsel")
            nc.sync.dma_start(out=sel8_sb[:], in_=sel8[:])
            nc.sync.dma_start(out=qsel_sb[:], in_=qsel[:])

            # nd_sb[b][g, 496*s16 + f] = -(dist_partial) for row(b, s16, g, f)
            nd_sb = [
                wpool.tile([8, 16 * FREE], mybir.dt.float32, tag=f"nd{b}", name=f"nd{b}")
                for b in range(2)
            ]
            nd_sp = [
                wpool.tile([128, FREE], mybir.dt.float32, tag=f"ndsp{b}", name=f"ndsp{b}")
                for b in range(2)
            ]
            dpd = [
                wpool.tile([128, FREE], mybir.dt.float32, tag=f"dpd{b}", name=f"dpd{b}")
                for b in range(2)
            ]
            wv = [
                wpool.tile([128, FREE], mybir.dt.float32, tag=f"wv{b}", name=f"wv{b}")
                for b in range(2)
            ]
            tk_in = [
                wpool.tile([16, 8 * FREE], mybir.dt.float32, tag=f"tkin{b}", name=f"tkin{b}")
                for b in range(2)
            ]
            tk_out = [
                wpool.tile([16, 32], mybir.dt.uint32, tag=f"tkout{b}", name=f"tkout{b}")
                for b in range(2)
            ]
            wacc_sb = wpool.tile([128, 2], mybir.dt.float32, tag="wacc")

            for t in range(NTILE):
                # tile t covers psum slices {2t, 2t+1}: per quarter, cols
                # [Q*QBLK + 2*FREE*t, Q*QBLK + 2*FREE*(t+1)).
                ktile = kpool.tile([128, 8 * FREE], mybir.dt.float32, tag="ktile")
                src = kt.rearrange("p (q u) -> p q u", q=4)[
                    :, :, 2 * FREE * t : 2 * FREE * (t + 1)
                ]
                nc.sync.dma_start(
                    out=ktile.rearrange("p (q u) -> p q u", q=4), in_=src
                )

                sq = sqpool.tile([128, 8 * FREE], mybir.dt.float32, tag="sq")
                nc.vector.tensor_tensor(
                    sq[:], ktile[:], ktile[:], mybir.AluOpType.mult
                )

                ps = [
                    psum.tile([8, FREE], mybir.dt.float32, tag="ps", name=f"ps{t}_{_}")
                    for _ in range(2)
                ]
                # -Sum of squares: 4 quarter matmuls per slice, -1 selector.
                for ss in range(2):
                    for Q in range(4):
                        c0 = 2 * FREE * Q + FREE * ss
                        nc.tensor.matmul(
                            ps[ss][:],
                            sel8_sb[:],
                            sq[:, c0 : c0 + FREE],
                            start=(Q == 0),
                            stop=False,
                        )
                # +2 q.k: 4 quarter matmuls per slice, +2q selector.
                for Q in range(4):
                    for ss in range(2):
                        c0 = 2 * FREE * Q + FREE * ss
                        nc.tensor.matmul(
                            ps[ss][:],
                            qsel_sb[:, 8 * Q : 8 * (Q + 1)],
                            ktile[:, c0 : c0 + FREE],
                            start=False,
                            stop=(Q == 3),
                        )
                # Evacuate -dist_partial from PSUM.
                for ss in range(2):
                    s = 2 * t + ss
                    b, s16 = divmod(s, 16)
                    nc.scalar.copy(
                        nd_sb[b][:, FREE * s16 : FREE * (s16 + 1)], ps[ss][:]
                    )

                if t in (NTILE // 2 - 1, NTILE - 1):
                    b = t // (NTILE // 2)
                    # Candidate path: tk_in[p16, 496*g + f] = nd_sb[g, 496*p16 + f]
                    # -> vocab v = 3968*p16 + 496*g + f = row - 63488*b.
                    for g in range(8):
                        nc.sync.dma_start(
                            out=tk_in[b][:, FREE * g : FREE * (g + 1)],
                            in_=nd_sb[b][g : g + 1, :],
                        )
                    _emit_topk(nc, tk_out[b][:], tk_in[b][:], 1, HALF, TOPK_K)
                    nc.sync.dma_start(
                        out=out_topk[:, 32 * b : 32 * (b + 1)], in_=tk_out[b][:]
                    )
                    # Partial-sum path: spread [8, 7936] -> [128, 496], then
                    # w = 1/(bias - nd) with exact DVE reciprocal, reduce.
                    for g in range(8):
                        nc.sync.dma_start(
                            out=nd_sp[b][16 * g : 16 * (g + 1), :],
                            in_=nd_sb[b][g : g + 1, :],
                        )
                    nc.vector.tensor_scalar(
                        out=dpd[b][:],
                        in0=nd_sp[b][:],
                        scalar1=-1.0,
                        scalar2=bias_const,
                        op0=mybir.AluOpType.mult,
                        op1=mybir.AluOpType.add,
                    )
                    nc.vector.reciprocal(wv[b][:], dpd[b][:])
                    nc.vector.tensor_reduce(
                        out=wacc_sb[:, b : b + 1],
                        in_=wv[b][:],
                        axis=mybir.AxisListType.X,
                        op=mybir.AluOpType.add,
                    )

            nc.sync.dma_start(out=out_wacc[:], in_=wacc_sb[:])

    nc.compile()
    return nc


def _host_inputs(q: np.ndarray, keys: np.ndarray):
    """Build the per-core DRAM input arrays."""
    # Selectors are negated so PSUM accumulates -(||k||^2 - 2 q.k).
    sel8 = np.zeros((128, 8), np.float32)
    for j in range(8):
        sel8[16 * j : 16 * (j + 1), j] = -1.0
    qsel = np.zeros((128, 32), np.float32)
    for j in range(8):
        for Q in range(4):
            qsel[16 * j : 16 * (j + 1), 8 * Q + j] = 2.0 * q[16 * Q : 16 * (Q + 1)]

    in_maps = []
    for c in range(NCORES):
        shard = keys[c * SHARD : (c + 1) * SHARD]
        pad = np.full((RPAD, D), PAD_VAL, np.float32)
        pad[:SHARD] = shard
        # [b, s16, g, f, Q, cq] -> [g, cq, Q, b, s16, f] -> [128, W]
        kt = np.ascontiguousarray(
            pad.reshape(2, 16, 8, FREE, 4, 16)
            .transpose(2, 5, 4, 0, 1, 3)
            .reshape(128, W)
        )
        in_maps.append({"kt": kt, "sel8": sel8, "qsel": qsel})
    return in_maps


def decode_rows(tk: np.ndarray, b: int) -> np.ndarray:
    """Decode topk call b's indices from out_topk [16, 64] to shard rows:
    row = 63488*b + v."""
    v = tk[:, 32 * b + 16 : 32 * b + 32].astype(np.int64).reshape(-1)
    return HALF * b + v


def _merge(results, q: np.ndarray, keys: np.ndarray, values: np.ndarray):
    """Host-side gather/unshard: exact top-50 over the candidate superset."""
    S = np.float32(
        sum(np.asarray(r["out_wacc"], np.float64).sum() for r in results)
    )
    g_list = []
    for c, r in enumerate(results):
        tk = np.asarray(r["out_topk"])  # [16, 64] uint32
        for b in range(2):
            rows = decode_rows(tk, b)
            rows = rows[rows < SHARD]
            g_list.append(c * SHARD + rows)
    g = np.unique(np.concatenate(g_list))
    # exact fp32 recompute of candidate weights
    diff = q[None, :] - keys[g]
    d = (diff * diff).sum(axis=1, dtype=np.float32)
    w = np.float32(1.0) / (d + np.float32(DELTA))
    order = np.lexsort((g, -w))  # descending w, ties by lower global index
    sel = order[:QUERY_WIDTH]
    weights = (w[sel] / S).astype(np.float32)[:, None]
    out = (values[g[sel]] * weights).sum(axis=0, keepdims=True, dtype=np.float32)
    return out.astype(np.float32)


_NC_CACHE: dict = {}


def _get_nc(bias_const: float):
    if bias_const not in _NC_CACHE:
        _NC_CACHE[bias_const] = _build_nc(bias_const)
    return _NC_CACHE[bias_const]


def kernel(key, keys, values):
    from concourse.bass_utils import run_bass_kernel_spmd

    q = np.ascontiguousarray(np.asarray(key, np.float32))
    K = np.ascontiguousarray(np.asarray(keys, np.float32))
    V = np.ascontiguousarray(np.asarray(values, np.float32))
    assert q.shape == (D,) and K.shape == (N_TOTAL, D) and V.shape == (N_TOTAL, D)

    bias_const = float(
        np.float32(DELTA) + (q.astype(np.float32) ** 2).sum(dtype=np.float32)
    )
    nc = _get_nc(bias_const)
    in_maps = _host_inputs(q, K)
    res = run_bass_kernel_spmd(nc, in_maps, list(range(NCORES))).results
    return _merge(res, q, K, V)


# revision 29
# speedup vs baseline: 3.4078x; 3.4078x over previous
"""Distributed k-NN retrieval kernel for Trainium2 (8 NeuronCores, SPMD).

Math (per the problem): w_i = 1 / (||q - k_i||^2 + delta) over 1M keys;
top-50 w; out = sum_j values[idx_j] * (w_j / sum_i w_i), shape [1, 64].

Strategy: shard keys row-wise across 8 cores (125000 rows each, padded to
126976 = 2 * 63488). Each core computes all shard NEGATED partial
distances nd = -(||k||^2 - 2 q.k) = -dist + ||q||^2 with bf16 tensor-
engine matmuls accumulating in fp32 PSUM (channels on partitions; top-k
of nd == top-k of w since w = 1/(dist + delta) is strictly decreasing in
dist; the bf16 input rounding perturbs dist by ~0.3 while the rank-50 vs
rank-256 distance margin on randn data is ~6.5, so the candidate
superset is safe, and final weights are recomputed exactly on the host).
Candidates: per 63488-row half, the nd values are spread to [128, 496]
and 3 rounds of DVE max8 / max_index / match_replace extract the top-24
per partition (6144 per core) -- a provable superset of the global
top-50 unless >24 of the top-50 land in one of the 256 (partition, half)
bins. The partial sum of w is computed exactly on-device from the same
spread: dist+delta recovered with one tensor_scalar, then DVE reciprocal
(iterative divide) + reduce. The host gathers candidate indices +
partial sums, recomputes candidate weights exactly in fp32, and does the
final top-50 weighted gather-sum (tiny: O(50k)).

Device-side layout (per core):
  row r in [0, 126976), decomposed r = 63488*b + 3968*s16 + 496*g + f
    b   in {0,1}   : half              (psum slice s = 16*b + s16)
    s16 in [0,16)  : psum-slice within half
    g   in [0,8)   : row group (psum partition)
    f   in [0,496) : psum free column
  channel c = 16*Q + cq (quarter Q in [0,4), cq in [0,16))
  kt[16*g + cq, 15872*Q + 496*s + f] = keys_pad[r, c]   (bf16)
  Each psum slice [8, 496] accumulates 8 matmuls (4 quarters x {-k^2, +2qk}).
  Spread: nd_sp[b][16*g + s16, f] = nd(row) -> candidate (b, p, j, v):
  row = 63488*b + 3968*(p % 16) + 496*(p // 16) + v.
"""

import sys

import numpy as np

for _p in ("/opt/trn_rl_repo", "/opt/pypackages"):
    if _p not in sys.path:
        sys.path.insert(0, _p)

DELTA = 0.001
QUERY_WIDTH = 50
N_TOTAL = 1_000_000
D = 64
NCORES = 8
SHARD = N_TOTAL // NCORES  # 125000
FREE = 496                 # psum free columns per slice
SROWS = 8 * FREE           # 3968 rows per psum slice
HALF = 16 * SROWS          # 63488 rows per half
RPAD = 2 * HALF            # 126976 padded rows per core
W = RPAD * D // 128        # 63488 columns of the transposed layout
QBLK = W // 4              # 15872 cols per channel-quarter block
NTILE = 16                 # DMA tiles, each covers 2 psum slices
NROUND = 3                 # max8 rounds -> top-24 per partition per half
PAD_VAL = 1.0e6


def _build_nc(bias_const: float):
    import concourse.bacc as bacc
    import concourse.mybir as mybir
    import concourse.tile as tile

    nc = bacc.Bacc(None, target_bir_lowering=False)

    kt = nc.dram_tensor("kt", [128, W], mybir.dt.bfloat16, kind="ExternalInput")
    sel8 = nc.dram_tensor("sel8", [128, 8], mybir.dt.bfloat16, kind="ExternalInput")
    q2 = nc.dram_tensor("q2", [128, 4], mybir.dt.float32, kind="ExternalInput")
    out_cand = nc.dram_tensor(
        "out_cand", [128, 2 * NROUND * 8], mybir.dt.uint32, kind="ExternalOutput"
    )
    out_wacc = nc.dram_tensor(
        "out_wacc", [128, 2], mybir.dt.float32, kind="ExternalOutput"
    )

    with tile.TileContext(nc) as tc:
        with (
            tc.tile_pool(name="consts", bufs=1) as consts,
            tc.tile_pool(name="kpool", bufs=3) as kpool,
            tc.tile_pool(name="sqpool", bufs=2) as sqpool,
            tc.tile_pool(name="wpool", bufs=1) as wpool,
            tc.tile_pool(name="psum", bufs=4, space="PSUM") as psum,
        ):
            sel8_sb = consts.tile([128, 8], mybir.dt.bfloat16, tag="sel8")
            q2_sb = consts.tile([128, 4], mybir.dt.float32, tag="q2")
            nc.sync.dma_start(out=sel8_sb[:], in_=sel8[:])
            nc.sync.dma_start(out=q2_sb[:], in_=q2[:])

            # nd_sb[b][g, 496*s16 + f] = -(dist_partial) for row(b, s16, g, f)
            nd_sb = [
                wpool.tile([8, 16 * FREE], mybir.dt.float32, tag=f"nd{b}", name=f"nd{b}")
                for b in range(2)
            ]
            nd_sp = [
                wpool.tile([128, FREE], mybir.dt.float32, tag=f"ndsp{b}", name=f"ndsp{b}")
                for b in range(2)
            ]
            dpd = [
                wpool.tile([128, FREE], mybir.dt.float32, tag=f"dpd{b}", name=f"dpd{b}")
                for b in range(2)
            ]
            wv = [
                wpool.tile([128, FREE], mybir.dt.float32, tag=f"wv{b}", name=f"wv{b}")
                for b in range(2)
            ]
            scr = [
                wpool.tile([128, FREE], mybir.dt.float32, tag=f"scr{i}", name=f"scr{i}")
                for i in range(2)
            ]
            mx = [
                wpool.tile([128, 8], mybir.dt.float32, tag=f"mx{i}", name=f"mx{i}")
                for i in range(NROUND)
            ]
            cand_sb = wpool.tile([128, 2 * NROUND * 8], mybir.dt.uint32, tag="cand")
            wacc_sb = wpool.tile([128, 2], mybir.dt.float32, tag="wacc")

            for t in range(NTILE):
                # tile t covers psum slices {2t, 2t+1}: per quarter, cols
                # [Q*QBLK + 2*FREE*t, Q*QBLK + 2*FREE*(t+1)).
                ktile = kpool.tile([128, 8 * FREE], mybir.dt.bfloat16, tag="ktile")
                src = kt.rearrange("p (q u) -> p q u", q=4)[
                    :, :, 2 * FREE * t : 2 * FREE * (t + 1)
                ]
                nc.sync.dma_start(
                    out=ktile.rearrange("p (q u) -> p q u", q=4), in_=src
                )

                # km2q = (k - 2q) * k = k^2 - 2 q.k, per quarter chunk
                # (the per-partition scalar 2q depends on the quarter).
                km2q = sqpool.tile([128, 8 * FREE], mybir.dt.bfloat16, tag="km2q")
                for Q in range(4):
                    c0 = 2 * FREE * Q
                    nc.vector.scalar_tensor_tensor(
                        out=km2q[:, c0 : c0 + 2 * FREE],
                        in0=ktile[:, c0 : c0 + 2 * FREE],
                        scalar=q2_sb[:, Q : Q + 1],
                        in1=ktile[:, c0 : c0 + 2 * FREE],
                        op0=mybir.AluOpType.subtract,
                        op1=mybir.AluOpType.mult,
                    )

                ps = [
                    psum.tile([8, FREE], mybir.dt.float32, tag="ps", name=f"ps{t}_{_}")
                    for _ in range(2)
                ]
                # nd = -sum(km2q): 4 quarter matmuls per slice, -1 selector.
                for ss in range(2):
                    for Q in range(4):
                        c0 = 2 * FREE * Q + FREE * ss
                        nc.tensor.matmul(
                            ps[ss][:],
                            sel8_sb[:],
                            km2q[:, c0 : c0 + FREE],
                            start=(Q == 0),
                            stop=(Q == 3),
                        )
                # Evacuate -dist_partial from PSUM (fp32).
                for ss in range(2):
                    s = 2 * t + ss
                    b, s16 = divmod(s, 16)
                    nc.scalar.copy(
                        nd_sb[b][:, FREE * s16 : FREE * (s16 + 1)], ps[ss][:]
                    )

                if t in (NTILE // 2 - 1, NTILE - 1):
                    b = t // (NTILE // 2)
                    # Spread [8, 7936] -> [128, 496]:
                    # nd_sp[16g + s16, f] = nd_sb[g, 496*s16 + f]
                    for g in range(8):
                        nc.sync.dma_start(
                            out=nd_sp[b][16 * g : 16 * (g + 1), :],
                            in_=nd_sb[b][g : g + 1, :],
                        )
                    # Candidate path: NROUND rounds of top-8-per-partition.
                    cur = nd_sp[b]
                    for r in range(NROUND):
                        nc.vector.max(mx[r][:], cur[:])
                        nc.vector.max_index(
                            cand_sb[:, 8 * (NROUND * b + r) : 8 * (NROUND * b + r + 1)],
                            mx[r][:],
                            cur[:],
                        )
                        if r < NROUND - 1:
                            nxt = scr[r % 2]
                            nc.vector.match_replace(
                                nxt[:], mx[r][:], cur[:], -1.0e30
                            )
                            cur = nxt
                    # Partial-sum path: w = 1/(bias - nd), exact DVE
                    # reciprocal, per-partition reduce.
                    nc.vector.tensor_scalar(
                        out=dpd[b][:],
                        in0=nd_sp[b][:],
                        scalar1=-1.0,
                        scalar2=bias_const,
                        op0=mybir.AluOpType.mult,
                        op1=mybir.AluOpType.add,
                    )
                    nc.vector.reciprocal(wv[b][:], dpd[b][:])
                    nc.vector.tensor_reduce(
                        out=wacc_sb[:, b : b + 1],
                        in_=wv[b][:],
                        axis=mybir.AxisListType.X,
                        op=mybir.AluOpType.add,
                    )

            nc.sync.dma_start(out=out_cand[:], in_=cand_sb[:])
            nc.sync.dma_start(out=out_wacc[:], in_=wacc_sb[:])

    nc.compile()
    return nc


def _host_inputs(q: np.ndarray, keys: np.ndarray):
    """Build the per-core DRAM input arrays (bf16 keys layout)."""
    import ml_dtypes

    bf16 = ml_dtypes.bfloat16
    # Selector is negated so PSUM accumulates -(||k||^2 - 2 q.k).
    sel8 = np.zeros((128, 8), bf16)
    for j in range(8):
        sel8[16 * j : 16 * (j + 1), j] = bf16(-1.0)
    # q2[p, Q] = 2*q[16*Q + p%16]: per-partition scalar for quarter Q.
    q2 = np.zeros((128, 4), np.float32)
    for Q in range(4):
        q2[:, Q] = np.tile(2.0 * q[16 * Q : 16 * (Q + 1)], 8)

    in_maps = []
    for c in range(NCORES):
        shard = keys[c * SHARD : (c + 1) * SHARD]
        pad = np.full((RPAD, D), PAD_VAL, np.float32)
        pad[:SHARD] = shard
        # [b, s16, g, f, Q, cq] -> [g, cq, Q, b, s16, f] -> [128, W]
        kt = np.ascontiguousarray(
            pad.reshape(2, 16, 8, FREE, 4, 16)
            .transpose(2, 5, 4, 0, 1, 3)
            .reshape(128, W)
            .astype(bf16)
        )
        in_maps.append({"kt": kt, "sel8": sel8, "q2": q2})
    return in_maps


def decode_rows(cand: np.ndarray, b: int) -> np.ndarray:
    """Decode half b's candidate indices from out_cand [128, 48] to shard
    rows. Candidate (p, round r, j) has value v in [0, 496):
    row = 63488*b + 3968*(p % 16) + 496*(p // 16) + v."""
    v = cand[:, 8 * NROUND * b : 8 * NROUND * (b + 1)].astype(np.int64)  # [128, 24]
    p = np.arange(128)[:, None]
    rows = HALF * b + SROWS * (p % 16) + FREE * (p // 16) + v
    # max_index emits -1 (wrapped to u32 max) for unmatched entries; out-of-
    # range v also lands outside the shard and is filtered by the caller.
    rows[(v < 0) | (v >= FREE)] = RPAD
    return rows.reshape(-1)


def _merge(results, q: np.ndarray, keys: np.ndarray, values: np.ndarray):
    """Host-side gather/unshard: exact top-50 over the candidate superset."""
    S = np.float32(
        sum(np.asarray(r["out_wacc"], np.float64).sum() for r in results)
    )
    g_list = []
    for c, r in enumerate(results):
        cand = np.asarray(r["out_cand"])  # [128, 48] uint32
        for b in range(2):
            rows = decode_rows(cand, b)
            rows = rows[rows < SHARD]
            g_list.append(c * SHARD + rows)
    g = np.unique(np.concatenate(g_list))
    # exact fp32 recompute of candidate weights
    diff = q[None, :] - keys[g]
    d = (diff * diff).sum(axis=1, dtype=np.float32)
    w = np.float32(1.0) / (d + np.float32(DELTA))
    order = np.lexsort((g, -w))  # descending w, ties by lower global index
    sel = order[:QUERY_WIDTH]
    weights = (w[sel] / S).astype(np.float32)[:, None]
    out = (values[g[sel]] * weights).sum(axis=0, keepdims=True, dtype=np.float32)
    return out.astype(np.float32)


_NC_CACHE: dict = {}


def _get_nc(bias_const: float):
    if bias_const not in _NC_CACHE:
        _NC_CACHE[bias_const] = _build_nc(bias_const)
    return _NC_CACHE[bias_const]


def kernel(key, keys, values):
    from concourse.bass_utils import run_bass_kernel_spmd

    q = np.ascontiguousarray(np.asarray(key, np.float32))
    K = np.ascontiguousarray(np.asarray(keys, np.float32))
    V = np.ascontiguousarray(np.asarray(values, np.float32))
    assert q.shape == (D,) and K.shape == (N_TOTAL, D) and V.shape == (N_TOTAL, D)

    bias_const = float(
        np.float32(DELTA) + (q.astype(np.float32) ** 2).sum(dtype=np.float32)
    )
    nc = _get_nc(bias_const)
    in_maps = _host_inputs(q, K)
    res = run_bass_kernel_spmd(nc, in_maps, list(range(NCORES))).results
    return _merge(res, q, K, V)
